# revision 1
# baseline (speedup 1.0000x reference)
"""HGATConv (4-head graph attention, N=4096, F=512) on 8 Trainium2 NeuronCores.

Sharding: node rows split across 8 cores (512 rows each). Each core:
  - computes S = x @ (W@A) and h = x @ W (fp32r matmuls: full PE rate vs
    4x-slow fp32) for its own rows,
  - packs one bf16 gather payload per 128-row block: aug-layout h
    [h0|1|h1|1|h2|1|h3|1] (the ones column makes the 129-wide weighted-sum
    matmul produce softmax denominators for free), sj bitcast as f32 pairs,
    and exp(sj/5),
  - AllGathers the payload in 4 row chunks (the first collective pays a
    one-time cross-core rendezvous; later chunks pipeline, so the main loop
    starts after chunk 0 instead of the full gather),
  - computes its (H, 512, 4096) attention rows in transposed layout
    [j=partitions, i=free] so the weighted sum runs directly on the PE,
  - writes its 512 output rows.

Score math: with s = si[i] + sj[j],
  exp(leakyrelu(s)) = max(e^s, e^(s/5)) = e^si * [max(1, e^(-.8si)e^(-.8sj))] * e^sj
The e^si factor cancels between softmax numerator and denominator, and
e^sj multiplies the gathered h rows (one ACT Copy with per-partition
scale per head), so the main loop does NO exponentials at all: just one
DVE tensor_scalar (max(1, E21*g)) per head plus one head-batched mask
multiply. That cut main-loop time ~35% beyond the engine-balance gain —
with 8.4M exps gone, the PE itself speeds up ~25% (less SBUF contention
and power-throttle headroom). Keeping Pool (gpsimd) idle during the main
loop matters for the same reason.
"""

import sys
import numpy as np

if "/opt/trn_rl_repo" not in sys.path:
    sys.path.insert(0, "/opt/trn_rl_repo")

H, D = 4, 128          # heads, head dim
N, F = 4096, 512       # nodes, features
M = 8                  # cores
NP = N // M            # 512 node rows per core
AUG = D + 1            # 129 (head dim + ones column)
CH = H * AUG           # 516 aug-h bf16 cols
CH2 = CH + 4 * H       # + f1=exp(sj), g=exp(-.8 sj) (4 f32 each, bitcast) = 532
JB = N // 128          # 32 j blocks
IB = NP // 128         # 4 i blocks
KB = F // 128          # 4 contraction blocks
ALPHA = 0.2

_CACHE = {}


def _build_nc():
    import concourse.bacc as bacc
    from concourse import mybir
    from concourse.tile import TileContext

    f32 = mybir.dt.float32
    f32r = mybir.dt.float32r
    bf16 = mybir.dt.bfloat16
    Alu = mybir.AluOpType
    Act = mybir.ActivationFunctionType

    nc = bacc.Bacc(num_swdge_queues=4)
    xT_d = nc.declare_dram_parameter("xT", [F, NP], f32r, isOutput=False)
    W_d = nc.declare_dram_parameter("W", [F, F], f32r, isOutput=False)
    WA_d = nc.declare_dram_parameter("WA", [F, 2 * H], f32r, isOutput=False)
    maskT_d = nc.declare_dram_parameter("maskT", [N, NP], bf16, isOutput=False)
    out_d = nc.declare_dram_parameter("out", [NP, F], f32, isOutput=True)

    with TileContext(nc) as tc:
        with (
            tc.tile_pool(name="const", bufs=1) as const_pool,
            tc.tile_pool(name="dram", bufs=1, space="DRAM") as dram_pool,
        ):
            cch_in = dram_pool.tile([NP, CH2], bf16)
            # chunked AllGather: chunk c gathers each core's rows
            # [c*128,(c+1)*128) -> [M*128, CH2]; row cc*128+r = core cc's row r.
            # Global j-block (cc*IB + c) lives in chunk c at rows cc*128..
            cch_out = [dram_pool.tile([M * 128, CH2], bf16, addr_space="Shared",
                                      name=f"cch_out_{c}")
                       for c in range(IB)]

            # ---- load inputs ----
            xT_sb = const_pool.tile([128, KB * NP], f32r)    # k-tiles side by side
            W_sb = const_pool.tile([128, KB * F], f32r)
            WA_sb = const_pool.tile([128, KB * 2 * H], f32r)
            for k in range(KB):
                nc.sync.dma_start(xT_sb[:, k * NP:(k + 1) * NP],
                                  xT_d[k * 128:(k + 1) * 128, :])
                nc.sync.dma_start(W_sb[:, k * F:(k + 1) * F],
                                  W_d[k * 128:(k + 1) * 128, :])
                nc.sync.dma_start(WA_sb[:, k * 2 * H:(k + 1) * 2 * H],
                                  WA_d[k * 128:(k + 1) * 128, :])

            si_b = const_pool.tile([128, H * NP], f32)    # si broadcast per head
            E21 = const_pool.tile([128, H * NP], bf16)    # exp(-0.8*si) per head
            sT_sb = const_pool.tile([8, NP], f32)         # rows 0..3 si, 4..7 sj
            sT_dram = dram_pool.tile([1, H * NP], f32)    # si rows staged in DRAM

            # ---- stage A: scores (f32) then h (f32r); gathers issued ASAP ----
            with (
                tc.tile_pool(name="apsum", bufs=2, space="PSUM") as apsum,
                tc.tile_pool(name="astage", bufs=2) as astage,
            ):
                for ib in range(IB):
                    ps = apsum.tile([128, 2 * H], f32, tag="ps")
                    for k in range(KB):
                        nc.tensor.matmul(
                            ps[:],
                            lhsT=xT_sb[:, k * NP + ib * 128: k * NP + (ib + 1) * 128],
                            rhs=WA_sb[:, k * 2 * H:(k + 1) * 2 * H],
                            start=(k == 0), stop=(k == KB - 1))
                    ph = apsum.tile([128, F], f32, tag="ph")
                    for k in range(KB):
                        nc.tensor.matmul(
                            ph[:],
                            lhsT=xT_sb[:, k * NP + ib * 128:
                                       k * NP + (ib + 1) * 128],
                            rhs=W_sb[:, k * F:(k + 1) * F],
                            start=(k == 0), stop=(k == KB - 1))
                    # pack [aug-h bf16 | f1=exp(sj) | g=exp(-0.8 sj)] (f32 bitcast)
                    hb = astage.tile([128, CH2], bf16, tag="hb")
                    hb3 = hb[:, 0:CH].rearrange("p (a c) -> p a c", c=AUG)
                    nc.scalar.activation(
                        hb3[:, :, 0:D],
                        ph[:].rearrange("p (a c) -> p a c", c=D), Act.Copy)
                    nc.vector.memset(hb3[:, :, D:AUG], 1.0)
                    nc.scalar.activation(
                        hb[:, CH:CH + 2 * H].bitcast(f32), ps[:, H:2 * H],
                        Act.Exp)
                    nc.scalar.activation(
                        hb[:, CH + 2 * H:CH + 4 * H].bitcast(f32),
                        ps[:, H:2 * H], Act.Exp, scale=-4.0 * ALPHA)
                    nc.sync.dma_start(cch_in[ib * 128:(ib + 1) * 128, :], hb[:])
                    nc.gpsimd.collective_compute(
                        "AllGather", mybir.AluOpType.bypass,
                        replica_groups=[list(range(M))],
                        ins=[cch_in[ib * 128:(ib + 1) * 128, :].opt()],
                        outs=[cch_out[ib][:].opt()])

                # E21 tiles (overlap the h AllGather wire time): replicate
                # si row h across partitions via a DRAM round-trip DMA with
                # a broadcast read AP — no gpsimd library dependency
                pst = apsum.tile([8, NP], f32, tag="pst")
                for k in range(KB):
                    nc.tensor.matmul(
                        pst[:],
                        lhsT=WA_sb[:, k * 2 * H:(k + 1) * 2 * H],
                        rhs=xT_sb[:, k * NP:(k + 1) * NP],
                        start=(k == 0), stop=(k == KB - 1))
                nc.vector.tensor_copy(sT_sb[:], pst[:])

                for h in range(H):
                    nc.sync.dma_start(sT_dram[0:1, h * NP:(h + 1) * NP],
                                      sT_sb[h:h + 1, :])
                    nc.sync.dma_start(
                        si_b[:, h * NP:(h + 1) * NP],
                        sT_dram[0:1, h * NP:(h + 1) * NP]
                        .partition_broadcast(128))
                    nc.scalar.activation(E21[:, h * NP:(h + 1) * NP],
                                         si_b[:, h * NP:(h + 1) * NP],
                                         Act.Exp, scale=-4.0 * ALPHA)

            # ---- main attention loop ----
            with (
                tc.tile_pool(name="acc", bufs=1, space="PSUM") as acc_pool,
                tc.tile_pool(name="stream", bufs=6) as stream,
                tc.tile_pool(name="pp", bufs=4) as pp,
                tc.tile_pool(name="tail", bufs=2) as tail_pool,
            ):
                # accumulators: per i-block, two tiles of [128, 2*129] (heads 0-1, 2-3)
                acc = [[acc_pool.tile([128, 2 * AUG], f32,
                                      name=f"acc_{ib}_{g}") for g in range(2)]
                       for ib in range(IB)]

                for jj in range(JB):
                    # consume chunks in arrival order: chunk c, then source core
                    c, cc = jj // M, jj % M
                    jb = cc * IB + c        # global j block
                    first, last = (jj == 0), (jj == JB - 1)
                    hs = stream.tile([128, CH2], bf16, tag="hs")
                    nc.sync.dma_start(hs[:], cch_out[c][cc * 128:(cc + 1) * 128, :])
                    f1 = hs[:, CH:CH + 2 * H].bitcast(f32)
                    gg = hs[:, CH + 2 * H:CH + 4 * H].bitcast(f32)
                    mask = stream.tile([128, NP], bf16, tag="mask")
                    nc.sync.dma_start(mask[:], maskT_d[jb * 128:(jb + 1) * 128, :])

                    # softmax factoring: pm = m*max(e^s, e^(s/5)) with
                    # s = si+sj splits as e^si * [m*max(1, e^(-.8si)e^(-.8sj))]
                    # * e^sj; e^si cancels in the softmax, e^sj rides the rhs.
                    rhs2 = stream.tile([128, CH], bf16, tag="rhs2")
                    pa = pp.tile([128, H * NP], bf16, tag="pa")
                    pm = pp.tile([128, H * NP], bf16, tag="pm")
                    for h in range(H):
                        nc.scalar.activation(
                            rhs2[:, h * AUG:(h + 1) * AUG],
                            hs[:, h * AUG:(h + 1) * AUG],
                            Act.Copy, scale=f1[:, h:h + 1])
                        nc.vector.tensor_scalar(
                            pa[:, h * NP:(h + 1) * NP],
                            in0=E21[:, h * NP:(h + 1) * NP],
                            scalar1=gg[:, h:h + 1], scalar2=1.0,
                            op0=Alu.mult, op1=Alu.max)
                    # masked weights for all 4 heads in one DVE op
                    nc.vector.tensor_tensor(
                        pm[:].rearrange("p (h n) -> p h n", h=H),
                        pa[:].rearrange("p (h n) -> p h n", h=H),
                        mask[:].unsqueeze(1).broadcast_to([128, H, NP]),
                        op=Alu.mult)

                    for h in range(H):
                        g, lh = divmod(h, 2)
                        for ib in range(IB):
                            # start=True clears the whole PSUM bank, so only
                            # the first head (lh==0) in each shared bank may
                            # set it; lh==1's first write lands on cleared
                            # has_written bits and overwrites.
                            nc.tensor.matmul(
                                acc[ib][g][:, lh * AUG:(lh + 1) * AUG],
                                lhsT=pm[:, h * NP + ib * 128:
                                        h * NP + (ib + 1) * 128],
                                rhs=rhs2[:, h * AUG:(h + 1) * AUG],
                                start=(first and lh == 0),
                                stop=(last and lh == 1),
                                skip_group_check=True)

                # ---- tail: normalize + elu + store ----
                for ib in range(IB):
                    rinv = tail_pool.tile([128, H], f32, tag="rinv")
                    for h in range(H):
                        g, lh = divmod(h, 2)
                        nc.vector.reciprocal(
                            rinv[:, h:h + 1],
                            acc[ib][g][:, lh * AUG + D: lh * AUG + D + 1])
                    # osb = acc * rinv via ACT Copy with per-partition scale
                    osb = tail_pool.tile([128, F], bf16, tag="osb")
                    for h in range(H):
                        g, lh = divmod(h, 2)
                        nc.scalar.activation(
                            osb[:, h * D:(h + 1) * D],
                            acc[ib][g][:, lh * AUG: lh * AUG + D],
                            Act.Copy, scale=rinv[:, h:h + 1])
                    # elu(x) = (relu(x) - 1) + exp(min(x, 0))
                    zmin = tail_pool.tile([128, F], bf16, tag="zmin")
                    nc.vector.tensor_scalar(zmin[:], in0=osb[:], scalar1=0.0,
                                            scalar2=None, op0=Alu.min)
                    ez = tail_pool.tile([128, F], f32, tag="ez")
                    nc.scalar.activation(ez[:], zmin[:], Act.Exp)
                    rm1 = tail_pool.tile([128, F], f32, tag="rm1")
                    nc.vector.tensor_scalar(rm1[:], in0=osb[:], scalar1=0.0,
                                            scalar2=-1.0, op0=Alu.max, op1=Alu.add)
                    oo = tail_pool.tile([128, F], f32, tag="oo")
                    nc.vector.tensor_tensor(oo[:], ez[:], rm1[:], op=Alu.add)
                    nc.sync.dma_start(out_d[ib * 128:(ib + 1) * 128, :], oo[:])

    nc.compile()
    return nc


def _host_prep(x, adj, W, a):
    x = np.ascontiguousarray(np.asarray(x, np.float32))
    adj = np.asarray(adj)
    W = np.ascontiguousarray(np.asarray(W, np.float32))
    a = np.asarray(a, np.float32)

    A = np.zeros((F, 2 * H), np.float32)
    for h in range(H):
        A[h * D:(h + 1) * D, h] = a[:D, 0]
        A[h * D:(h + 1) * D, H + h] = a[D:, 0]
    WA = np.ascontiguousarray(W @ A)

    import ml_dtypes
    xT = np.ascontiguousarray(x.T)
    adjT = np.ascontiguousarray(adj.T.astype(ml_dtypes.bfloat16))

    in_maps = []
    for c in range(M):
        cols = slice(c * NP, (c + 1) * NP)
        in_maps.append({
            "xT": np.ascontiguousarray(xT[:, cols]),
            "W": W,
            "WA": WA,
            "maskT": np.ascontiguousarray(adjT[:, cols]),
        })
    return in_maps


def kernel(x, adj, W, a):
    from concourse.bass_utils import run_bass_kernel_spmd

    if "nc" not in _CACHE:
        _CACHE["nc"] = _build_nc()
    nc = _CACHE["nc"]

    in_maps = _host_prep(x, adj, W, a)
    res = run_bass_kernel_spmd(nc, in_maps, list(range(M)))
    outs = [np.asarray(r["out"], np.float32) for r in res.results]
    return np.concatenate(outs, axis=0)


if __name__ == "__main__":
    nc = _build_nc()
    print("built ok")

